# revision 3
# baseline (speedup 1.0000x reference)
"""Trainium2 Bass kernel for nn_GaussianDecoder.

Contract: kernel(**inputs) takes FULL unsharded inputs (numpy) and returns the
FULL (N, 11) output. Internally:
  - host: shard N across 8 cores, transpose activations to [feat, token],
    pre-transpose/pad weights, fold FiLM into per-partition scale/bias.
  - device (x8 NeuronCores): the 4 matmul layers with fused FiLM/ReLU,
    producing out^T = [11, tokens] per core.
  - host: epilogue (softplus/sigmoid/tanh/quaternion fusion incl. 4x4 eigh)
    mirrored op-for-op with jax on CPU so the eigh sign convention matches the
    reference.

Self-contained: shapes/sharding hardcoded, no sibling imports.
"""

import numpy as np

# problem shapes (hardcoded)
IN_DIM, HIDDEN, OUT_DIM, Z_DIM = 515, 256, 11, 64
B, N = 2, 131072
NCORES = 8
NS = N // NCORES            # 16384 points per core
TOK = B * NS                # 32768 token rows per core
TT = 512                    # token tile (fp32 moving-operand max / one PSUM bank)
NTILES = TOK // TT          # 64
KC_FULL = IN_DIM // 128     # 4 full 128-row chunks
KC_TAIL = IN_DIM - KC_FULL * 128  # 3
IN_PAD = (KC_FULL + 1) * 128      # 640

_CACHE = {}


def _build_nc(compute_dt_name="float32r"):
    import concourse.bacc as bacc
    import concourse.mybir as mybir
    from concourse.tile import TileContext

    f32 = mybir.dt.float32
    cdt = getattr(mybir.dt, compute_dt_name)
    nc = bacc.Bacc("TRN2", target_bir_lowering=False, debug=True)

    xt = nc.dram_tensor("xt", [IN_DIM, TOK], cdt, kind="ExternalInput")
    fc1w = nc.dram_tensor("fc1w", [IN_PAD, HIDDEN], cdt, kind="ExternalInput")
    w1t = nc.dram_tensor("w1t", [HIDDEN, HIDDEN], cdt, kind="ExternalInput")
    w2t = nc.dram_tensor("w2t", [HIDDEN, HIDDEN], cdt, kind="ExternalInput")
    w3t = nc.dram_tensor("w3t", [HIDDEN, OUT_DIM], cdt, kind="ExternalInput")
    # per-partition scale/bias consts: cols 0-3 scale1[b*2+m], 4-7 bias1[b*2+m],
    # 8-9 bias_l2[m], 10-11 bias_l3[m]
    sb = nc.dram_tensor("sb", [128, 12], f32, kind="ExternalInput")
    outd = nc.dram_tensor("out", [OUT_DIM, TOK], f32, kind="ExternalOutput")

    Relu = mybir.ActivationFunctionType.Relu
    mx = mybir.AluOpType.max
    add = mybir.AluOpType.add

    with TileContext(nc) as tc:
        with (
            tc.tile_pool(name="const", bufs=1) as cp,
            tc.tile_pool(name="work", bufs=3) as wp,
            tc.tile_pool(name="hact", bufs=2) as hp,
            tc.tile_pool(name="ps", bufs=2, space="PSUM") as pp,
        ):
            fc1sb = cp.tile([128, KC_FULL + 1, HIDDEN], cdt)
            nc.sync.dma_start(fc1sb[:], fc1w.rearrange("(c p) h -> p c h", p=128))
            w1sb = cp.tile([128, 2, HIDDEN], cdt)
            nc.sync.dma_start(w1sb[:], w1t.rearrange("(c p) h -> p c h", p=128))
            w2sb = cp.tile([128, 2, HIDDEN], cdt)
            nc.sync.dma_start(w2sb[:], w2t.rearrange("(c p) h -> p c h", p=128))
            w3sb = cp.tile([128, 2, OUT_DIM], cdt)
            nc.sync.dma_start(w3sb[:], w3t.rearrange("(c p) o -> p c o", p=128))
            sbsb = cp.tile([128, 12], f32)
            nc.sync.dma_start(sbsb[:], sb[:])

            for t in range(NTILES):
                b = t // (NTILES // B)
                t0 = t * TT
                xsb = wp.tile([128, KC_FULL + 1, TT], cdt, tag="x")
                nc.sync.dma_start(
                    xsb[:, :KC_FULL, :],
                    xt[: KC_FULL * 128, t0 : t0 + TT].rearrange(
                        "(c p) t -> p c t", p=128
                    ),
                )
                nc.sync.dma_start(
                    xsb[:KC_TAIL, KC_FULL, :],
                    xt[KC_FULL * 128 :, t0 : t0 + TT],
                )

                # L1: FiLM-modulated first layer
                h1 = hp.tile([128, 2, TT], cdt, tag="h1")
                for m in range(2):
                    ps1 = pp.tile([128, TT], f32, tag="ps1")
                    for c in range(KC_FULL + 1):
                        kk = 128 if c < KC_FULL else KC_TAIL
                        nc.tensor.matmul(
                            ps1[:],
                            fc1sb[:kk, c, m * 128 : (m + 1) * 128],
                            xsb[:kk, c, :],
                            start=(c == 0),
                            stop=(c == KC_FULL),
                        )
                    col = b * 2 + m
                    nc.scalar.activation(
                        h1[:, m, :], ps1[:], Relu,
                        bias=sbsb[:, 4 + col : 5 + col],
                        scale=sbsb[:, col : col + 1],
                    )

                # L2 (relu(x@w1.T+b1)) on DVE
                h2 = hp.tile([128, 2, TT], cdt, tag="h2")
                for m in range(2):
                    ps2 = pp.tile([128, TT], f32, tag="ps2")
                    for k in range(2):
                        nc.tensor.matmul(
                            ps2[:],
                            w1sb[:, k, m * 128 : (m + 1) * 128],
                            h1[:, k, :],
                            start=(k == 0),
                            stop=(k == 1),
                        )
                    nc.vector.tensor_scalar(
                        h2[:, m, :], ps2[:],
                        sbsb[:, 8 + m : 9 + m], 0.0, add, mx,
                    )

                # L3 on ACT
                h3 = hp.tile([128, 2, TT], cdt, tag="h3")
                for m in range(2):
                    ps3 = pp.tile([128, TT], f32, tag="ps3")
                    for k in range(2):
                        nc.tensor.matmul(
                            ps3[:],
                            w2sb[:, k, m * 128 : (m + 1) * 128],
                            h2[:, k, :],
                            start=(k == 0),
                            stop=(k == 1),
                        )
                    nc.scalar.activation(
                        h3[:, m, :], ps3[:], Relu,
                        bias=sbsb[:, 10 + m : 11 + m],
                        scale=1.0,
                    )

                # L4: out^T tile [11, TT] (b3 added on host)
                ps4 = pp.tile([OUT_DIM, TT], f32, tag="ps4")
                for k in range(2):
                    nc.tensor.matmul(
                        ps4[:],
                        w3sb[:, k, :],
                        h3[:, k, :],
                        start=(k == 0),
                        stop=(k == 1),
                    )
                o4 = wp.tile([OUT_DIM, TT], f32, tag="o4")
                nc.vector.tensor_copy(o4[:], ps4[:])
                nc.sync.dma_start(outd[:, t0 : t0 + TT], o4[:])

    nc.finalize()
    return nc


def _get_nc():
    key = "nc"
    if key not in _CACHE:
        _CACHE[key] = _build_nc()
    return _CACHE[key]


def _host_prep(combined_feats, z_id, fc1_w, fc1_b, film_w, film_b, w1, b1, w2, b2, w3):
    f = np.float32
    gb = z_id.astype(f) @ film_w.astype(f).T + film_b.astype(f)
    gamma, beta = gb[:, :HIDDEN], gb[:, HIDDEN:]
    scale1 = (1.0 + gamma).astype(f)                      # (B, H)
    bias1 = (scale1 * fc1_b.astype(f) + beta).astype(f)   # (B, H)

    sb = np.zeros((128, 12), f)
    for b in range(B):
        for m in range(2):
            sb[:, b * 2 + m] = scale1[b, m * 128 : (m + 1) * 128]
            sb[:, 4 + b * 2 + m] = bias1[b, m * 128 : (m + 1) * 128]
    for m in range(2):
        sb[:, 8 + m] = b1[m * 128 : (m + 1) * 128]
        sb[:, 10 + m] = b2[m * 128 : (m + 1) * 128]

    fc1wT = np.zeros((IN_PAD, HIDDEN), f)
    fc1wT[:IN_DIM] = fc1_w.astype(f).T
    w1T = np.ascontiguousarray(w1.astype(f).T)
    w2T = np.ascontiguousarray(w2.astype(f).T)
    w3T = np.ascontiguousarray(w3.astype(f).T)

    in_maps = []
    for c in range(NCORES):
        n0 = c * NS
        # [515, B*NS] with column index = b*NS + n_local
        xtc = np.ascontiguousarray(
            combined_feats[:, n0 : n0 + NS, :].transpose(2, 0, 1).reshape(IN_DIM, TOK)
        )
        in_maps.append(
            {"xt": xtc, "fc1w": fc1wT, "w1t": w1T, "w2t": w2T, "w3t": w3T, "sb": sb}
        )
    return in_maps


def _host_epilogue(out_bn11):
    """Mirror the reference epilogue op-for-op with jax on CPU (eigh sign
    convention must match the reference's LAPACK build)."""
    import jax
    import jax.numpy as jnp

    cpu = jax.devices("cpu")[0]
    with jax.default_device(cpu):
        out = jnp.asarray(out_bn11)
        eps = 1e-6
        scales = jnp.clip(jax.nn.softplus(out[..., 0:3]) + eps, 1e-6, 3.0)
        rot_raw = out[..., 3:7]
        rot = rot_raw / (jnp.linalg.norm(rot_raw, axis=-1, keepdims=True) + 1e-8)
        alpha = jnp.clip(jax.nn.sigmoid(out[..., 7]), 1e-6, 1.0)
        sh = jnp.tanh(out[..., 8:]) * 0.5
        w = jnp.clip(alpha, 0.0, 1.0)
        w = w / jnp.maximum(w.sum(axis=0, keepdims=True), 1e-8)
        scales_agg = jnp.einsum("bn,bnk->nk", w, scales)
        sh_agg = jnp.einsum("bn,bnk->nk", w, sh)
        M = jnp.einsum("bn,bni,bnj->nij", w, rot, rot)
        _, eigvecs = jnp.linalg.eigh(M)
        avg_q = eigvecs[..., -1]
        avg_q = avg_q / (jnp.linalg.norm(avg_q, axis=-1, keepdims=True) + 1e-12)
        alpha_mean = (w * alpha).sum(axis=0)
        res = jnp.concatenate(
            [scales_agg, avg_q, alpha_mean[:, None], sh_agg], axis=-1
        )
        return np.asarray(res)


def _install_ntff_hook():
    """Dev-only (KERNEL_TRACE=1): register the axon NTFF profile hook that
    this image's antenv package lacks, so trace=True works."""
    import sys, types
    name = "antenv.axon_hooks"
    if name in sys.modules:
        return
    mod = types.ModuleType(name)
    _hook = [None]
    mod.set_axon_ntff_profile_hook = lambda h: _hook.__setitem__(0, h)
    mod.get_axon_ntff_profile_hook = lambda: _hook[0]
    sys.modules[name] = mod
    import antenv
    antenv.axon_hooks = mod
    from trn_agent_boot.trn_boot import _ntff_profile_via_ctypes
    mod.set_axon_ntff_profile_hook(
        _ntff_profile_via_ctypes("/opt/axon/libaxon_pjrt.so")
    )


def kernel(combined_feats, z_id, fc1_w, fc1_b, film_w, film_b,
           w1, b1, w2, b2, w3, b3, **_unused):
    import os
    from concourse.bass_utils import run_bass_kernel_spmd

    combined_feats = np.asarray(combined_feats, dtype=np.float32)
    in_maps = _host_prep(
        np.asarray(combined_feats), np.asarray(z_id), np.asarray(fc1_w),
        np.asarray(fc1_b), np.asarray(film_w), np.asarray(film_b),
        np.asarray(w1), np.asarray(b1), np.asarray(w2), np.asarray(b2),
        np.asarray(w3),
    )
    nc = _get_nc()
    kwargs = {}
    if os.environ.get("KERNEL_TRACE"):
        _install_ntff_hook()
        kwargs = dict(trace=True, tmpdir=os.environ.get("KERNEL_TRACE_DIR") or None)
    res = run_bass_kernel_spmd(nc, in_maps, list(range(NCORES)), **kwargs)
    _CACHE["last_results"] = res

    # gather: core c holds out^T [11, B*NS]; assemble (B, N, 11)
    out = np.empty((B, N, OUT_DIM), np.float32)
    for c in range(NCORES):
        o = np.asarray(res.results[c]["out"])  # [11, TOK]
        o = o.reshape(OUT_DIM, B, NS).transpose(1, 2, 0)  # (B, NS, 11)
        out[:, c * NS : (c + 1) * NS, :] = o
    out += np.asarray(b3, np.float32)

    return _host_epilogue(out)


# revision 4
# speedup vs baseline: 1.0064x; 1.0064x over previous
"""Trainium2 Bass kernel for nn_GaussianDecoder.

Contract: kernel(**inputs) takes FULL unsharded inputs (numpy) and returns the
FULL (N, 11) output. Internally:
  - host: shard N across 8 cores, transpose activations to [feat, token],
    pre-transpose/pad weights, fold FiLM into per-partition scale/bias.
  - device (x8 NeuronCores): the 4 matmul layers with fused FiLM/ReLU,
    producing out^T = [11, tokens] per core.
  - host: epilogue (softplus/sigmoid/tanh/quaternion fusion incl. 4x4 eigh)
    mirrored op-for-op with jax on CPU so the eigh sign convention matches the
    reference.

Self-contained: shapes/sharding hardcoded, no sibling imports.
"""

import numpy as np

# problem shapes (hardcoded)
IN_DIM, HIDDEN, OUT_DIM, Z_DIM = 515, 256, 11, 64
B, N = 2, 131072
NCORES = 8
NS = N // NCORES            # 16384 points per core
TOK = B * NS                # 32768 token rows per core
TT = 512                    # token tile (fp32 moving-operand max / one PSUM bank)
NTILES = TOK // TT          # 64
KC_FULL = IN_DIM // 128     # 4 full 128-row chunks
KC_TAIL = IN_DIM - KC_FULL * 128  # 3
IN_PAD = (KC_FULL + 1) * 128      # 640

_CACHE = {}


def _build_nc(compute_dt_name="float32r"):
    import concourse.bacc as bacc
    import concourse.mybir as mybir
    from concourse.tile import TileContext

    f32 = mybir.dt.float32
    cdt = getattr(mybir.dt, compute_dt_name)
    nc = bacc.Bacc("TRN2", target_bir_lowering=False, debug=True)

    xt = nc.dram_tensor("xt", [IN_DIM, TOK], cdt, kind="ExternalInput")
    fc1w = nc.dram_tensor("fc1w", [IN_PAD, HIDDEN], cdt, kind="ExternalInput")
    w1t = nc.dram_tensor("w1t", [HIDDEN, HIDDEN], cdt, kind="ExternalInput")
    w2t = nc.dram_tensor("w2t", [HIDDEN, HIDDEN], cdt, kind="ExternalInput")
    w3t = nc.dram_tensor("w3t", [HIDDEN, OUT_DIM], cdt, kind="ExternalInput")
    # per-partition scale/bias consts: cols 0-3 scale1[b*2+m], 4-7 bias1[b*2+m],
    # 8-9 bias_l2[m], 10-11 bias_l3[m]
    sb = nc.dram_tensor("sb", [128, 12], f32, kind="ExternalInput")
    outd = nc.dram_tensor("out", [OUT_DIM, TOK], f32, kind="ExternalOutput")

    Relu = mybir.ActivationFunctionType.Relu
    mx = mybir.AluOpType.max
    add = mybir.AluOpType.add

    with TileContext(nc) as tc:
        with (
            tc.tile_pool(name="const", bufs=1) as cp,
            tc.tile_pool(name="work", bufs=5) as wp,
            tc.tile_pool(name="hact", bufs=3) as hp,
            tc.tile_pool(name="ps", bufs=2, space="PSUM") as pp,
        ):
            fc1sb = cp.tile([128, KC_FULL + 1, HIDDEN], cdt)
            nc.sync.dma_start(fc1sb[:], fc1w.rearrange("(c p) h -> p c h", p=128))
            w1sb = cp.tile([128, 2, HIDDEN], cdt)
            nc.sync.dma_start(w1sb[:], w1t.rearrange("(c p) h -> p c h", p=128))
            w2sb = cp.tile([128, 2, HIDDEN], cdt)
            nc.sync.dma_start(w2sb[:], w2t.rearrange("(c p) h -> p c h", p=128))
            w3sb = cp.tile([128, 2, OUT_DIM], cdt)
            nc.sync.dma_start(w3sb[:], w3t.rearrange("(c p) o -> p c o", p=128))
            sbsb = cp.tile([128, 12], f32)
            nc.sync.dma_start(sbsb[:], sb[:])

            for t in range(NTILES):
                b = t // (NTILES // B)
                t0 = t * TT
                xsb = wp.tile([128, KC_FULL + 1, TT], cdt, tag="x")
                nc.sync.dma_start(
                    xsb[:, :KC_FULL, :],
                    xt[: KC_FULL * 128, t0 : t0 + TT].rearrange(
                        "(c p) t -> p c t", p=128
                    ),
                )
                nc.sync.dma_start(
                    xsb[:KC_TAIL, KC_FULL, :],
                    xt[KC_FULL * 128 :, t0 : t0 + TT],
                )

                # L1: FiLM-modulated first layer
                h1 = hp.tile([128, 2, TT], cdt, tag="h1")
                for m in range(2):
                    ps1 = pp.tile([128, TT], f32, tag="ps1")
                    for c in range(KC_FULL + 1):
                        kk = 128 if c < KC_FULL else KC_TAIL
                        nc.tensor.matmul(
                            ps1[:],
                            fc1sb[:kk, c, m * 128 : (m + 1) * 128],
                            xsb[:kk, c, :],
                            start=(c == 0),
                            stop=(c == KC_FULL),
                        )
                    col = b * 2 + m
                    nc.scalar.activation(
                        h1[:, m, :], ps1[:], Relu,
                        bias=sbsb[:, 4 + col : 5 + col],
                        scale=sbsb[:, col : col + 1],
                    )

                # L2 (relu(x@w1.T+b1)) on DVE
                h2 = hp.tile([128, 2, TT], cdt, tag="h2")
                for m in range(2):
                    ps2 = pp.tile([128, TT], f32, tag="ps2")
                    for k in range(2):
                        nc.tensor.matmul(
                            ps2[:],
                            w1sb[:, k, m * 128 : (m + 1) * 128],
                            h1[:, k, :],
                            start=(k == 0),
                            stop=(k == 1),
                        )
                    nc.vector.tensor_scalar(
                        h2[:, m, :], ps2[:],
                        sbsb[:, 8 + m : 9 + m], 0.0, add, mx,
                    )

                # L3 on ACT
                h3 = hp.tile([128, 2, TT], cdt, tag="h3")
                for m in range(2):
                    ps3 = pp.tile([128, TT], f32, tag="ps3")
                    for k in range(2):
                        nc.tensor.matmul(
                            ps3[:],
                            w2sb[:, k, m * 128 : (m + 1) * 128],
                            h2[:, k, :],
                            start=(k == 0),
                            stop=(k == 1),
                        )
                    nc.scalar.activation(
                        h3[:, m, :], ps3[:], Relu,
                        bias=sbsb[:, 10 + m : 11 + m],
                        scale=1.0,
                    )

                # L4: out^T tile [11, TT] (b3 added on host)
                ps4 = pp.tile([OUT_DIM, TT], f32, tag="ps4")
                for k in range(2):
                    nc.tensor.matmul(
                        ps4[:],
                        w3sb[:, k, :],
                        h3[:, k, :],
                        start=(k == 0),
                        stop=(k == 1),
                    )
                o4 = wp.tile([OUT_DIM, TT], f32, tag="o4")
                nc.vector.tensor_copy(o4[:], ps4[:])
                nc.sync.dma_start(outd[:, t0 : t0 + TT], o4[:])

    nc.finalize()
    return nc


def _get_nc():
    key = "nc"
    if key not in _CACHE:
        _CACHE[key] = _build_nc()
    return _CACHE[key]


def _host_prep(combined_feats, z_id, fc1_w, fc1_b, film_w, film_b, w1, b1, w2, b2, w3):
    f = np.float32
    gb = z_id.astype(f) @ film_w.astype(f).T + film_b.astype(f)
    gamma, beta = gb[:, :HIDDEN], gb[:, HIDDEN:]
    scale1 = (1.0 + gamma).astype(f)                      # (B, H)
    bias1 = (scale1 * fc1_b.astype(f) + beta).astype(f)   # (B, H)

    sb = np.zeros((128, 12), f)
    for b in range(B):
        for m in range(2):
            sb[:, b * 2 + m] = scale1[b, m * 128 : (m + 1) * 128]
            sb[:, 4 + b * 2 + m] = bias1[b, m * 128 : (m + 1) * 128]
    for m in range(2):
        sb[:, 8 + m] = b1[m * 128 : (m + 1) * 128]
        sb[:, 10 + m] = b2[m * 128 : (m + 1) * 128]

    fc1wT = np.zeros((IN_PAD, HIDDEN), f)
    fc1wT[:IN_DIM] = fc1_w.astype(f).T
    w1T = np.ascontiguousarray(w1.astype(f).T)
    w2T = np.ascontiguousarray(w2.astype(f).T)
    w3T = np.ascontiguousarray(w3.astype(f).T)

    in_maps = []
    for c in range(NCORES):
        n0 = c * NS
        # [515, B*NS] with column index = b*NS + n_local
        xtc = np.ascontiguousarray(
            combined_feats[:, n0 : n0 + NS, :].transpose(2, 0, 1).reshape(IN_DIM, TOK)
        )
        in_maps.append(
            {"xt": xtc, "fc1w": fc1wT, "w1t": w1T, "w2t": w2T, "w3t": w3T, "sb": sb}
        )
    return in_maps


def _host_epilogue(out_bn11):
    """Mirror the reference epilogue op-for-op with jax on CPU (eigh sign
    convention must match the reference's LAPACK build)."""
    import jax
    import jax.numpy as jnp

    cpu = jax.devices("cpu")[0]
    with jax.default_device(cpu):
        out = jnp.asarray(out_bn11)
        eps = 1e-6
        scales = jnp.clip(jax.nn.softplus(out[..., 0:3]) + eps, 1e-6, 3.0)
        rot_raw = out[..., 3:7]
        rot = rot_raw / (jnp.linalg.norm(rot_raw, axis=-1, keepdims=True) + 1e-8)
        alpha = jnp.clip(jax.nn.sigmoid(out[..., 7]), 1e-6, 1.0)
        sh = jnp.tanh(out[..., 8:]) * 0.5
        w = jnp.clip(alpha, 0.0, 1.0)
        w = w / jnp.maximum(w.sum(axis=0, keepdims=True), 1e-8)
        scales_agg = jnp.einsum("bn,bnk->nk", w, scales)
        sh_agg = jnp.einsum("bn,bnk->nk", w, sh)
        M = jnp.einsum("bn,bni,bnj->nij", w, rot, rot)
        _, eigvecs = jnp.linalg.eigh(M)
        avg_q = eigvecs[..., -1]
        avg_q = avg_q / (jnp.linalg.norm(avg_q, axis=-1, keepdims=True) + 1e-12)
        alpha_mean = (w * alpha).sum(axis=0)
        res = jnp.concatenate(
            [scales_agg, avg_q, alpha_mean[:, None], sh_agg], axis=-1
        )
        return np.asarray(res)


def _install_ntff_hook():
    """Dev-only (KERNEL_TRACE=1): register the axon NTFF profile hook that
    this image's antenv package lacks, so trace=True works."""
    import sys, types
    name = "antenv.axon_hooks"
    if name in sys.modules:
        return
    mod = types.ModuleType(name)
    _hook = [None]
    mod.set_axon_ntff_profile_hook = lambda h: _hook.__setitem__(0, h)
    mod.get_axon_ntff_profile_hook = lambda: _hook[0]
    sys.modules[name] = mod
    import antenv
    antenv.axon_hooks = mod
    from trn_agent_boot.trn_boot import _ntff_profile_via_ctypes
    mod.set_axon_ntff_profile_hook(
        _ntff_profile_via_ctypes("/opt/axon/libaxon_pjrt.so")
    )


def kernel(combined_feats, z_id, fc1_w, fc1_b, film_w, film_b,
           w1, b1, w2, b2, w3, b3, **_unused):
    import os
    from concourse.bass_utils import run_bass_kernel_spmd

    combined_feats = np.asarray(combined_feats, dtype=np.float32)
    in_maps = _host_prep(
        np.asarray(combined_feats), np.asarray(z_id), np.asarray(fc1_w),
        np.asarray(fc1_b), np.asarray(film_w), np.asarray(film_b),
        np.asarray(w1), np.asarray(b1), np.asarray(w2), np.asarray(b2),
        np.asarray(w3),
    )
    nc = _get_nc()
    kwargs = {}
    if os.environ.get("KERNEL_TRACE"):
        _install_ntff_hook()
        kwargs = dict(trace=True, tmpdir=os.environ.get("KERNEL_TRACE_DIR") or None)
    res = run_bass_kernel_spmd(nc, in_maps, list(range(NCORES)), **kwargs)
    _CACHE["last_results"] = res

    # gather: core c holds out^T [11, B*NS]; assemble (B, N, 11)
    out = np.empty((B, N, OUT_DIM), np.float32)
    for c in range(NCORES):
        o = np.asarray(res.results[c]["out"])  # [11, TOK]
        o = o.reshape(OUT_DIM, B, NS).transpose(1, 2, 0)  # (B, NS, 11)
        out[:, c * NS : (c + 1) * NS, :] = o
    out += np.asarray(b3, np.float32)

    return _host_epilogue(out)
